# revision 3
# baseline (speedup 1.0000x reference)
"""Trainium2 Bass kernel for sheaf Dirichlet energy (ConsistencyBasedLaplacianBuilder).

loss = sum_e || maps[rev(e)] @ x[tgt(e)] - maps[e] @ x[src(e)] ||_F^2

Strategy (edge parallelism across 8 NeuronCores):
  The reference edge set is symmetric: edge e < H (=E/2) has its reverse at
  e + H, so the loss equals 2 * sum_{e<H} ||maps[e+H] x[dst] - maps[e] x[src]||^2.
  Each core takes a contiguous slice of the H half-edges, keeps a full replica
  of x in HBM, preloads its maps slice into SBUF, gathers x rows via indirect
  DMA, and does the 4x4 @ 4x16 contractions on the vector engine with
  per-partition-scalar MACs (128 edges per tile, one edge per partition).
  Per-core partial sums are added on the host.
"""

import sys
import types

import numpy as np

sys.path.insert(0, "/opt/trn_rl_repo")

N = 50000
D = 4
F = 16
DF = D * F            # 64 floats per node row
E = 1600000
H = E // 2            # 800000 undirected pairs
NCORES = 8
EPC = H // NCORES     # 100000 half-edges per core

GROUP = 8             # tiles gathered per indirect DMA
PAIR = 2 * GROUP      # tiles per loop iteration (double-buffered)
NT_USED = 784         # tiles per core (784*128 = 100352 >= 100000)
NT_ALLOC = 800        # padded columns (overhang gather reads into padding)
EPC_PAD = NT_USED * 128


def _inject_axon_hooks():
    """The container's antenv lacks axon_hooks; provide it so NTFF tracing
    (used by test.py, harmless otherwise) can register."""
    if "antenv.axon_hooks" in sys.modules:
        return
    mod = types.ModuleType("antenv.axon_hooks")
    mod._hook = None

    def set_axon_ntff_profile_hook(h):
        mod._hook = h

    def get_axon_ntff_profile_hook():
        return mod._hook

    mod.set_axon_ntff_profile_hook = set_axon_ntff_profile_hook
    mod.get_axon_ntff_profile_hook = get_axon_ntff_profile_hook
    sys.modules["antenv.axon_hooks"] = mod


def _build_program(nt_used=NT_USED, nt_alloc=NT_ALLOC, n_nodes=N, ncores=NCORES):
    import concourse.bacc as bacc
    import concourse.bass as bass
    import concourse.tile as tile
    from concourse import mybir

    f32 = mybir.dt.float32
    i32 = mybir.dt.int32
    Op = mybir.AluOpType
    ds = bass.ds

    ngroups = nt_used // GROUP
    assert ngroups % 2 == 0
    niters = ngroups // 2

    nc = bacc.Bacc("TRN2", target_bir_lowering=False, debug=False,
                   num_devices=ncores)

    x_d = nc.dram_tensor("x", [n_nodes, DF], f32, kind="ExternalInput")
    mlo_d = nc.dram_tensor("mlo", [128, nt_alloc * 16], f32, kind="ExternalInput")
    mhi_d = nc.dram_tensor("mhi", [128, nt_alloc * 16], f32, kind="ExternalInput")
    sidx_d = nc.dram_tensor("sidx", [128, nt_alloc], i32, kind="ExternalInput")
    didx_d = nc.dram_tensor("didx", [128, nt_alloc], i32, kind="ExternalInput")
    loss_d = nc.dram_tensor("loss", [1, 1], f32, kind="ExternalOutput")

    with tile.TileContext(nc) as tc, \
         tc.tile_pool(name="persist", bufs=1) as pp, \
         tc.tile_pool(name="gather", bufs=1) as gp, \
         tc.tile_pool(name="work", bufs=2) as wp, \
         tc.tile_pool(name="psum", bufs=1, space="PSUM") as psp:

        mlo_sb = pp.tile([128, nt_alloc * 16], f32, tag="mlo")
        mhi_sb = pp.tile([128, nt_alloc * 16], f32, tag="mhi")
        sidx_sb = pp.tile([128, nt_alloc], i32, tag="sidx")
        didx_sb = pp.tile([128, nt_alloc], i32, tag="didx")
        acc = pp.tile([128, nt_used], f32, tag="acc")

        nc.sync.dma_start(mlo_sb[:], mlo_d[:])
        nc.sync.dma_start(mhi_sb[:], mhi_d[:])
        nc.sync.dma_start(sidx_sb[:], sidx_d[:])
        nc.sync.dma_start(didx_sb[:], didx_d[:])

        # two static gather buffers (double buffer across the unrolled pair)
        xt_a = gp.tile([128, GROUP * DF], f32, tag="xt_a")
        xs_a = gp.tile([128, GROUP * DF], f32, tag="xs_a")
        xt_b = gp.tile([128, GROUP * DF], f32, tag="xt_b")
        xs_b = gp.tile([128, GROUP * DF], f32, tag="xs_b")
        # staging tiles: the indirect-DMA offset AP must have a static
        # address, so copy the (dynamically sliced) index columns here first
        st_d_a = gp.tile([128, GROUP], i32, tag="st_d_a")
        st_s_a = gp.tile([128, GROUP], i32, tag="st_s_a")
        st_d_b = gp.tile([128, GROUP], i32, tag="st_d_b")
        st_s_b = gp.tile([128, GROUP], i32, tag="st_s_b")

        def gather(tile0, xt_t, xs_t, st_d, st_s):
            # tile0: first tile index (RuntimeValue or int) of the GROUP
            nc.vector.tensor_copy(st_d[:], didx_sb[:, ds(tile0, GROUP)])
            nc.vector.tensor_copy(st_s[:], sidx_sb[:, ds(tile0, GROUP)])
            nc.gpsimd.indirect_dma_start(
                out=xt_t[:],
                out_offset=None,
                in_=x_d[:],
                in_offset=bass.IndirectOffsetOnAxis(ap=st_d[:], axis=0),
            )
            nc.gpsimd.indirect_dma_start(
                out=xs_t[:],
                out_offset=None,
                in_=x_d[:],
                in_offset=bass.IndirectOffsetOnAxis(ap=st_s[:], axis=0),
            )

        def compute(tile0, xt_t, xs_t):
            mhi_g = mhi_sb[:, ds(tile0 * 16, GROUP * 16)]
            mlo_g = mlo_sb[:, ds(tile0 * 16, GROUP * 16)]
            acc_g = acc[:, ds(tile0, GROUP)]
            for k in range(GROUP):
                d1 = wp.tile([128, DF], f32, tag="d1")
                d2 = wp.tile([128, DF], f32, tag="d2")
                sq = wp.tile([128, DF], f32, tag="sq")
                for term, (xg, mg, dd) in enumerate(
                        ((xt_t, mhi_g, d1), (xs_t, mlo_g, d2))):
                    for i in range(D):
                        o = dd[:, F * i:F * (i + 1)]
                        c0 = k * 16 + 4 * i
                        nc.vector.tensor_scalar(
                            o, xg[:, DF * k:DF * k + F],
                            mg[:, c0:c0 + 1], None, Op.mult)
                        for j in range(1, D):
                            nc.vector.scalar_tensor_tensor(
                                o, xg[:, DF * k + F * j:DF * k + F * (j + 1)],
                                mg[:, c0 + j:c0 + j + 1], o,
                                Op.mult, Op.add)
                nc.vector.tensor_tensor(d1[:], d1[:], d2[:], Op.subtract)
                nc.vector.scalar_tensor_tensor(
                    sq[:], d1[:], 0.0, d1[:], Op.bypass, Op.mult,
                    accum_out=acc_g[:, k:k + 1])

        gather(0, xt_a, xs_a, st_d_a, st_s_a)
        with tc.For_i(0, niters, 1,
                      hint_engines=(mybir.EngineType.DVE,)) as it:
            base = it * PAIR
            gather(base + GROUP, xt_b, xs_b, st_d_b, st_s_b)
            compute(base, xt_a, xs_a)
            gather(base + PAIR, xt_a, xs_a, st_d_a, st_s_a)
            compute(base + GROUP, xt_b, xs_b)

        colsum = pp.tile([128, 1], f32, tag="colsum")
        ones = pp.tile([128, 1], f32, tag="ones")
        nc.vector.reduce_sum(out=colsum[:], in_=acc[:],
                             axis=mybir.AxisListType.X)
        nc.gpsimd.memset(ones[:], 1.0)
        pt = psp.tile([1, 1], f32, tag="pt")
        nc.tensor.matmul(pt[:], lhsT=colsum[:], rhs=ones[:],
                         start=True, stop=True)
        lsb = pp.tile([1, 1], f32, tag="lsb")
        # *2: each undirected pair contributes both directed edges equally
        nc.vector.tensor_scalar(lsb[:], pt[:], 2.0, None, Op.mult)
        nc.sync.dma_start(loss_d[:], lsb[:])

    nc.compile()
    return nc


_CACHED = {}


def _get_program():
    if "nc" not in _CACHED:
        _inject_axon_hooks()
        _CACHED["nc"] = _build_program()
    return _CACHED["nc"]


def _prep_core_inputs(x_flat, maps2d, src, dst, core):
    """Build the per-core input dict (layout transforms only)."""
    e0 = core * EPC
    e1 = e0 + EPC

    mlo = np.zeros((128, NT_ALLOC * 16), np.float32)
    mhi = np.zeros((128, NT_ALLOC * 16), np.float32)
    sl = np.zeros((EPC_PAD, 16), np.float32)
    sl[:EPC] = maps2d[e0:e1]
    mlo[:, :NT_USED * 16] = (
        sl.reshape(NT_USED, 128, 16).transpose(1, 0, 2).reshape(128, -1))
    sl = np.zeros((EPC_PAD, 16), np.float32)
    sl[:EPC] = maps2d[H + e0:H + e1]
    mhi[:, :NT_USED * 16] = (
        sl.reshape(NT_USED, 128, 16).transpose(1, 0, 2).reshape(128, -1))

    sidx = np.zeros((128, NT_ALLOC), np.int32)
    didx = np.zeros((128, NT_ALLOC), np.int32)
    pad = np.zeros(EPC_PAD, np.int32)
    pad[:EPC] = src[e0:e1]
    sidx[:, :NT_USED] = pad.reshape(NT_USED, 128).T
    pad = np.zeros(EPC_PAD, np.int32)
    pad[:EPC] = dst[e0:e1]
    didx[:, :NT_USED] = pad.reshape(NT_USED, 128).T

    return {
        "x": x_flat,
        "mlo": np.ascontiguousarray(mlo),
        "mhi": np.ascontiguousarray(mhi),
        "sidx": np.ascontiguousarray(sidx),
        "didx": np.ascontiguousarray(didx),
    }


def _symmetric_structure(rev_idx):
    r = np.asarray(rev_idx)
    if r.shape != (E,):
        return False
    h = np.arange(H, dtype=r.dtype)
    return bool(np.array_equal(r[:H], h + H) and np.array_equal(r[H:], h))


def _fallback_numpy(x, restriction_maps, edge_index, rev_idx):
    x = np.asarray(x, np.float32)
    maps = np.asarray(restriction_maps, np.float32)
    ei = np.asarray(edge_index)
    rv = np.asarray(rev_idx)
    total = np.float64(0.0)
    chunk = 131072
    ne = ei.shape[1]
    for s in range(0, ne, chunk):
        e = min(s + chunk, ne)
        src = ei[0, s:e]
        tgt = ei[1, s:e]
        fvu = maps[rv[s:e]]
        fuv = maps[s:e]
        t1 = np.einsum("eij,ejf->eif", fvu, x[tgt])
        t2 = np.einsum("eij,ejf->eif", fuv, x[src])
        d = t1 - t2
        total += np.sum((d * d).astype(np.float64))
    return np.float32(total)


def kernel(x, restriction_maps, edge_index, rev_idx):
    x = np.asarray(x)
    restriction_maps = np.asarray(restriction_maps)
    edge_index = np.asarray(edge_index)
    rev_idx = np.asarray(rev_idx)

    if (x.shape != (N, D, F) or restriction_maps.shape != (E, D, D)
            or edge_index.shape != (2, E) or not _symmetric_structure(rev_idx)):
        return _fallback_numpy(x, restriction_maps, edge_index, rev_idx)

    from concourse.bass_utils import run_bass_kernel_spmd

    nc = _get_program()

    x_flat = np.ascontiguousarray(x.reshape(N, DF).astype(np.float32))
    maps2d = restriction_maps.reshape(E, 16).astype(np.float32)
    src = edge_index[0].astype(np.int32)
    dst = edge_index[1].astype(np.int32)

    in_maps = [_prep_core_inputs(x_flat, maps2d, src, dst, c)
               for c in range(NCORES)]
    res = run_bass_kernel_spmd(nc, in_maps, core_ids=list(range(NCORES)))
    total = np.float32(0.0)
    for c in range(NCORES):
        total += res.results[c]["loss"][0, 0]
    return np.float32(total)
